# revision 1
# baseline (speedup 1.0000x reference)
"""Causal self-attention (GQA 16q/4kv, rms_norm + rope + q_gain) on 8 trn2 cores.

Sharding: tensor-parallel over heads. Core c owns q-heads {2c, 2c+1} and kv-head
c//2. Each core computes its heads' attention output y_h^T [128, S] and a
partial projection  partial_c^T = Wproj[:, cols_c].T^T-contracted  [1024, S];
the host sums the 8 partials and transposes back to [1, S, 1024].

Layout strategy (per core, everything "d-major" = feature dim on partitions):
  Qt  [128, S] f32  rows = 2 heads x 64 dims  (rms-normed, roped, gained)
  Kt2 [128, S] f32  kv head duplicated to both partition halves (row-packed St)
  V   [128, 80*KB] bf16  seq-major tiles [128, 65] (col 64 = ones -> softmax sums)
  St block = Kt_blk.T @ Qt_chunk -> [128 k, 512 q] psum (2 heads row-packed)
  exp on ACT (scale=1/8) -> bf16, causal mask multiply on diagonal blocks
  AV: y^T[65, 512] += Vones_blk.T @ St_exp  (row 64 = sum of exp = softmax denom)
  normalize via DVE reciprocal + gpsimd partition_broadcast
  proj: partial^T[o-tile, q] = WprojT_tile.T @ y^T  -> DMA out
"""

import sys

for _p in ("/opt/trn_rl_repo",):
    if _p not in sys.path:
        sys.path.insert(0, _p)

import numpy as np
from contextlib import ExitStack

import concourse.bass as bass
import concourse.tile as tile
from concourse import mybir
from concourse.bass_utils import run_bass_kernel_spmd

F32 = mybir.dt.float32
F32R = mybir.dt.float32r
BF16 = mybir.dt.bfloat16
AF = mybir.ActivationFunctionType

DIM = 1024
NUM_HEADS = 16
NUM_KV_HEADS = 4
HD = 64
ROPE_BASE = 10000.0
RMS_EPS = 1.1920929e-07
N_CORES = 8

SCALE = 1.0 / 8.0  # 1/sqrt(64)


def build_nc(S: int, split: bool = True, iters: int = 1) -> bass.Bass:
    """Build the per-core SPMD Bass program for sequence length S (mult of 512)."""
    assert S % 512 == 0
    NCH = S // 512          # 512-wide seq chunks
    NKB = S // 128          # 128-wide k blocks

    nc = bass.Bass("TRN2", debug=False)

    xt_d = nc.declare_dram_parameter("xt", [DIM, S], F32R, isOutput=False)
    wqkv_d = nc.declare_dram_parameter("wqkv_t", [DIM, 256], F32R, isOutput=False)
    wproj_d = nc.declare_dram_parameter("wproj_t", [128, DIM], F32R, isOutput=False)
    cos_d = nc.declare_dram_parameter("cos4", [128, S], F32, isOutput=False)
    sin_d = nc.declare_dram_parameter("sin4s", [128, S], F32, isOutput=False)
    gain_d = nc.declare_dram_parameter("gain", [128, 1], F32, isOutput=False)
    out_d = nc.declare_dram_parameter("out", [DIM, S], F32, isOutput=True)
    vt_dram = nc.dram_tensor("vt_scratch", [64, S], BF16)

    with tile.TileContext(nc) as tc, ExitStack() as ctx:
        res = ctx.enter_context(tc.tile_pool(name="res", bufs=1))
        xtp = ctx.enter_context(tc.tile_pool(name="xtp", bufs=2))
        ropep = ctx.enter_context(tc.tile_pool(name="ropep", bufs=2))
        rowp = ctx.enter_context(tc.tile_pool(name="rowp", bufs=4))
        vtp = ctx.enter_context(tc.tile_pool(name="vtp", bufs=2))
        sep = ctx.enter_context(tc.tile_pool(name="sep", bufs=3))
        outp = ctx.enter_context(tc.tile_pool(name="outp", bufs=1))
        pst = ctx.enter_context(tc.tile_pool(name="pst", bufs=2, space="PSUM"))
        pyp = ctx.enter_context(tc.tile_pool(name="pyp", bufs=1, space="PSUM"))
        pms = ctx.enter_context(tc.tile_pool(name="pms", bufs=2, space="PSUM"))

        # ---- resident tiles ----
        qt_sb = res.tile([128, S], F32R, tag="qt", name="qt_sb")
        kt2_sb = res.tile([128, S], F32R, tag="kt2", name="kt2_sb")
        yt_sb = res.tile([128, S], F32R, tag="yt", name="yt_sb")
        v_sb = res.tile([128, NKB, 80], BF16, tag="v", name="v_sb")
        wqkv_sb = res.tile([128, 8, 256], F32R, tag="wqkv", name="wqkv_sb")
        wproj_sb = res.tile([128, DIM], F32R, tag="wproj", name="wproj_sb")
        cos_sb = res.tile([128, S], F32, tag="cos", name="cos_sb")
        sin_sb = res.tile([128, S], F32, tag="sin", name="sin_sb")
        gain_sb = res.tile([128, 1], F32, tag="gain", name="gain_sb")
        ones_m = res.tile([128, 64], BF16, tag="onesm", name="ones_m")
        ones_r = res.tile([1, 64], F32, tag="onesr", name="ones_r")
        ones_sb = res.tile([128, 1], F32, tag="ones", name="ones_sb")
        const_sb = res.tile([128, 2], F32, tag="const", name="const_sb")
        fbq = res.tile([128, 512], F32, tag="fbq", name="fbq")
        fbk = res.tile([64, 512], F32, tag="fbk", name="fbk")
        rr0 = res.tile([64, 512], F32, tag="rb0", name="rr0")
        rr1 = res.tile([64, 512], F32, tag="rb1", name="rr1")

        # ---- one-time setup ----
        for dt in range(8):
            nc.sync.dma_start(out=wqkv_sb[:, dt, :], in_=wqkv_d[dt * 128:(dt + 1) * 128, :])
        nc.sync.dma_start(out=wproj_sb[:], in_=wproj_d[:])
        nc.sync.dma_start(out=cos_sb[:], in_=cos_d[:])
        nc.sync.dma_start(out=sin_sb[:], in_=sin_d[:])
        nc.sync.dma_start(out=gain_sb[:], in_=gain_d[:])
        nc.vector.memset(ones_sb[:], 1.0)
        nc.vector.memset(ones_m[:], 1.0)
        nc.vector.memset(ones_r[:], 1.0)
        nc.vector.memset(const_sb[:, 0:1], 0.0)
        nc.vector.memset(const_sb[:, 1:2], RMS_EPS)
        nc.vector.memset(fbq[:], 0.0)
        nc.vector.memset(fbk[:], 0.0)
        nc.vector.memset(rr0[:], 0.0)
        nc.vector.memset(rr1[:], 0.0)
        zb128 = const_sb[:, 0:1]          # zero bias, 128 partitions
        zb64 = const_sb[0:64, 0:1]
        epsb = const_sb[0:1, 1:2]         # rms eps bias, 1 partition
        # ones column of every V tile (col 64 of each 80-wide block)
        nc.vector.memset(v_sb[:, :, 64:65], 1.0)

        def _chunks():
            def qkv_phase(c):
                s0 = c * 512
                sl = slice(s0, s0 + 512)

                # ---------- QKV projection (d-major): contraction over DIM ----------
                xtile = xtp.tile([128, 8, 512], F32R, tag="xt", name="xtile")
                xt_src = bass.AP(xt_d[:].tensor, s0, [[S, 128], [128 * S, 8], [1, 512]])
                nc.sync.dma_start(out=xtile[:], in_=xt_src)
                qkv_ps = pst.tile([128, 1024], F32, tag="st", name="qkv_ps")
                for dt in range(8):
                    nc.tensor.matmul(qkv_ps[:, 0:512], lhsT=wqkv_sb[:, dt, 0:128],
                                     rhs=xtile[:, dt, :], start=(dt == 0), stop=(dt == 7))
                for dt in range(8):
                    nc.tensor.matmul(qkv_ps[:, 512:1024], lhsT=wqkv_sb[:, dt, 128:256],
                                     rhs=xtile[:, dt, :], start=(dt == 0), stop=(dt == 7))

                # ---------- rms_norm factors ----------
                sq_q = ropep.tile([128, 512], BF16, tag="sqq", name="sq_q")
                nc.scalar.activation(sq_q[:], qkv_ps[:, 0:512], AF.Square, bias=zb128)
                sq_k = ropep.tile([64, 512], BF16, tag="sqk", name="sq_k")
                nc.scalar.activation(sq_k[:], qkv_ps[0:64, 512:1024], AF.Square, bias=zb64)

                msp_t = pms.tile([128, 512], F32, tag="b1", name="msp_t")
                nc.tensor.matmul(msp_t[0:64, :], lhsT=ones_m[0:64, :], rhs=sq_q[0:64, :],
                                 start=True, stop=True)
                nc.tensor.matmul(msp_t[64:128, :], lhsT=ones_m[64:128, :], rhs=sq_q[64:128, :],
                                 start=True, stop=True, tile_position=(64, 64))
                msk_t = pms.tile([128, 512], F32, tag="b1", name="msk_t")
                nc.tensor.matmul(msk_t[0:64, :], lhsT=ones_m[0:64, :], rhs=sq_k[:],
                                 start=True, stop=True)

                # rsqrt factors broadcast along partitions via stream_shuffle +
                # quadrant-aligned copies (no gpsimd extended ISA needed)
                nc.scalar.activation(fbq[:], msp_t[:], AF.Sqrt, bias=const_sb[:, 1:2], scale=1.0 / HD)
                nc.vector.reciprocal(fbq[:], fbq[:])
                nc.vector.tensor_scalar_mul(fbq[:], fbq[:], gain_sb[:, 0:1])
                nc.scalar.activation(fbk[:], msk_t[0:64, :], AF.Sqrt, bias=const_sb[0:64, 1:2], scale=1.0 / HD)
                nc.vector.reciprocal(fbk[:], fbk[:])

                # ---------- rope + norm, q (2 heads, 128 partitions) ----------
                qsw = ropep.tile([128, 512], F32, tag="qsw", name="qsw")
                for qd in range(4):
                    nc.vector.tensor_copy(qsw[32 * qd:32 * qd + 32, :],
                                          qkv_ps[32 * (qd ^ 1):32 * (qd ^ 1) + 32, 0:512])
                t1 = ropep.tile([128, 512], F32, tag="t1", name="t1")
                nc.vector.tensor_mul(t1[:], qkv_ps[:, 0:512], cos_sb[:, sl])
                t2 = ropep.tile([128, 512], F32, tag="t2", name="t2")
                nc.vector.tensor_mul(t2[:], qsw[:], sin_sb[:, sl])
                nc.vector.tensor_add(t1[:], t1[:], t2[:])
                nc.vector.tensor_mul(qt_sb[:, sl], t1[:], fbq[:])

                # ---------- rope + norm, k (1 kv head, 64 partitions) ----------
                ksw = ropep.tile([64, 512], F32, tag="ksw", name="ksw")
                nc.vector.tensor_copy(ksw[0:32, :], qkv_ps[32:64, 512:1024])
                nc.vector.tensor_copy(ksw[32:64, :], qkv_ps[0:32, 512:1024])
                kt1 = ropep.tile([64, 512], F32, tag="kt1", name="kt1")
                nc.vector.tensor_mul(kt1[:], qkv_ps[0:64, 512:1024], cos_sb[0:64, sl])
                kt2t = ropep.tile([64, 512], F32, tag="kt2t", name="kt2t")
                nc.vector.tensor_mul(kt2t[:], ksw[:], sin_sb[0:64, sl])
                nc.vector.tensor_add(kt1[:], kt1[:], kt2t[:])
                nc.vector.tensor_mul(kt2_sb[0:64, sl], kt1[:], fbk[:])
                # duplicate kv head to partitions 64..127 (quadrant-aligned moves)
                nc.vector.tensor_copy(kt2_sb[64:96, sl], kt2_sb[0:32, sl])
                nc.vector.tensor_copy(kt2_sb[96:128, sl], kt2_sb[32:64, sl])

                # ---------- V: bf16, transpose to seq-major via HBM bounce ----------
                vtt = vtp.tile([64, 512], BF16, tag="vt", name="vtt")
                nc.vector.tensor_copy(vtt[:], qkv_ps[64:128, 512:1024])
                nc.sync.dma_start(out=vt_dram[:, sl], in_=vtt[:])
                for j in range(4):
                    kb = 4 * c + j
                    nc.sync.dma_start_transpose(out=v_sb[:, kb, 0:64],
                                                in_=vt_dram[:, kb * 128:(kb + 1) * 128])

            def attn_phase(qc):
                s0 = qc * 512
                sl = slice(s0, s0 + 512)
                nkb = 4 * (qc + 1)
                y0 = pyp.tile([65, 512], F32, tag="y0", name="y0")
                y1 = pyp.tile([65, 512], F32, tag="y1", name="y1")
                for kb in range(nkb):
                    stp = pst.tile([128, 1024], F32, tag="st", name="stp")
                    nc.tensor.matmul(stp[:, 0:512], lhsT=kt2_sb[0:64, kb * 128:(kb + 1) * 128],
                                     rhs=qt_sb[0:64, sl], start=True, stop=True,
                                     tile_position=(0, 0))
                    nc.tensor.matmul(stp[:, 512:1024], lhsT=kt2_sb[64:128, kb * 128:(kb + 1) * 128],
                                     rhs=qt_sb[64:128, sl], start=True, stop=True,
                                     tile_position=(64, 0))
                    se = sep.tile([128, 1024], BF16, tag="se", name="se")
                    nc.scalar.activation(se[:], stp[:], AF.Exp, bias=zb128, scale=SCALE)
                    j = kb - 4 * qc
                    if j >= 0:  # diagonal block: zero the non-causal scores (idle Pool engine)
                        se3 = bass.AP(se.tensor, se.offset, [se.ap[0], [512, 2], [1, 512]])
                        nc.gpsimd.affine_select(
                            out=se3, in_=se3, compare_op=mybir.AluOpType.is_ge,
                            fill=0.0, base=-128 * j, pattern=[[0, 2], [1, 512]],
                            channel_multiplier=-1)
                    nc.tensor.matmul(y0[:], lhsT=v_sb[:, kb, 0:65], rhs=se[:, 0:512],
                                     start=(kb == 0), stop=(kb == nkb - 1))
                    nc.tensor.matmul(y1[:], lhsT=v_sb[:, kb, 0:65], rhs=se[:, 512:1024],
                                     start=(kb == 0), stop=(kb == nkb - 1))

                # ---------- softmax normalize -> yt ----------
                s0row = rowp.tile([1, 512], F32, tag="row", name="s0row")
                nc.vector.tensor_copy(s0row[:], y0[64:65, :])
                rps0 = pms.tile([64, 512], F32, tag="b1", name="rps0")
                nc.tensor.matmul(rps0[:], lhsT=ones_r[:], rhs=s0row[:], start=True, stop=True)
                nc.vector.reciprocal(rr0[0:64, :], rps0[:])
                nc.vector.tensor_mul(yt_sb[0:64, sl], y0[0:64, :], rr0[0:64, :])
                s1row = rowp.tile([1, 512], F32, tag="row", name="s1row")
                nc.vector.tensor_copy(s1row[:], y1[64:65, :])
                rps1 = pms.tile([64, 512], F32, tag="b1", name="rps1")
                nc.tensor.matmul(rps1[:], lhsT=ones_r[:], rhs=s1row[:], start=True, stop=True)
                nc.vector.reciprocal(rr1[0:64, :], rps1[:])
                # h1 write crosses partition quadrants: 32-wide quadrant-aligned moves
                nc.vector.tensor_mul(yt_sb[64:96, sl], y1[0:32, :], rr1[0:32, :])
                nc.vector.tensor_mul(yt_sb[96:128, sl], y1[32:64, :], rr1[32:64, :])

                # ---------- partial projection for this q-chunk ----------
                ot_big = outp.tile([128, 8, 512], F32, tag="ot", name="ot_big")
                for ot in range(8):
                    po = pms.tile([128, 512], F32, tag="b1", name="po")
                    nc.tensor.matmul(po[:], lhsT=wproj_sb[:, ot * 128:(ot + 1) * 128],
                                     rhs=yt_sb[:, sl], start=True, stop=True)
                    nc.vector.tensor_copy(ot_big[:, ot, :], po[:])
                out_dst = bass.AP(out_d[:].tensor, s0, [[S, 128], [128 * S, 8], [1, 512]])
                nc.sync.dma_start(out=out_dst, in_=ot_big[:])

            # lookahead-1 pipeline: emit QKV(c+1) before attention(c) so the
            # next chunk's projections/rope overlap the current attention
            qkv_phase(0)
            for c in range(1, NCH):
                qkv_phase(c)
                attn_phase(c - 1)
            attn_phase(NCH - 1)

        if iters > 1:
            with tc.For_i(0, iters, 1) as _i:
                _chunks()
        else:
            _chunks()

    if split:
        split_multi_waits(nc)
    return nc


def split_multi_waits(nc, max_waits=1):
    """walrus's per-instruction sync encoding only fits one sem wait on some
    instruction types (e.g. the matmul LDWEIGHTS struct). Hoist extra waits
    onto same-engine NoOps inserted just before the instruction."""
    nid = [0]
    for fn in nc.m.functions:
        for blk in fn.blocks:
            out = []
            for inst in blk.instructions:
                si = inst.sync_info
                if si is not None and len(si.on_wait) > max_waits:
                    waits = list(si.on_wait)
                    for w in waits[:-max_waits]:
                        nop = mybir.InstNoOp(name=f"waitsplit-{nid[0]}", ins=[], outs=[])
                        nid[0] += 1
                        nop.engine = inst.engine
                        nop.sync_info = mybir.SyncInfo(on_wait=[w], on_update=[])
                        out.append(nop)
                    inst.sync_info = mybir.SyncInfo(on_wait=waits[-max_waits:],
                                                    on_update=list(si.on_update))
                out.append(inst)
            blk.instructions = out


def make_host_inputs(x, Wq, Wk, Wv, Wproj, q_gain, S):
    """Slice/transpose full inputs into per-core in_maps (host-side prep)."""
    xt = np.ascontiguousarray(x.reshape(S, DIM).T).astype(np.float32, copy=False)

    inv_freq = 1.0 / (ROPE_BASE ** (np.arange(0, HD, 2, dtype=np.float32) / HD))
    t = np.arange(S, dtype=np.float32)
    freqs = np.outer(t, inv_freq).astype(np.float32)        # [S, 32]
    cos_t = np.cos(freqs).T.astype(np.float32)              # [32, S]
    sin_t = np.sin(freqs).T.astype(np.float32)
    cos4 = np.ascontiguousarray(np.tile(cos_t, (4, 1)))     # [128, S]
    sin4s = np.ascontiguousarray(
        np.concatenate([sin_t, -sin_t, sin_t, -sin_t], axis=0))  # [128, S]

    in_maps = []
    for c in range(N_CORES):
        kv = c // 2
        wq_c = Wq[128 * c:128 * (c + 1), :]                 # [128, 1024]
        wk_c = Wk[64 * kv:64 * (kv + 1), :]                 # [64, 1024]
        wv_c = Wv[64 * kv:64 * (kv + 1), :]
        wqkv_t = np.ascontiguousarray(
            np.concatenate([wq_c, wk_c, wv_c], axis=0).T).astype(np.float32, copy=False)
        wproj_t = np.ascontiguousarray(
            Wproj[:, 128 * c:128 * (c + 1)].T).astype(np.float32, copy=False)
        gain = np.ascontiguousarray(
            np.repeat(q_gain[2 * c:2 * c + 2], 64).reshape(128, 1)).astype(np.float32, copy=False)
        in_maps.append({
            "xt": xt,
            "wqkv_t": wqkv_t,
            "wproj_t": wproj_t,
            "cos4": cos4,
            "sin4s": sin4s,
            "gain": gain,
        })
    return in_maps


_NC_CACHE = {}


def get_nc(S):
    if S not in _NC_CACHE:
        _NC_CACHE[S] = build_nc(S)
    return _NC_CACHE[S]


def kernel(x, Wq, Wk, Wv, Wproj, q_gain, trace=False):
    x = np.asarray(x, dtype=np.float32)
    B, S, D = x.shape
    assert B == 1 and D == DIM
    in_maps = make_host_inputs(
        x, np.asarray(Wq, np.float32), np.asarray(Wk, np.float32),
        np.asarray(Wv, np.float32), np.asarray(Wproj, np.float32),
        np.asarray(q_gain, np.float32), S)

    nc = get_nc(S)
    r = run_bass_kernel_spmd(nc, in_maps, core_ids=list(range(N_CORES)), trace=trace)
    total = np.zeros((DIM, S), dtype=np.float32)
    for c in range(N_CORES):
        total += r.results[c]["out"]
    out = np.ascontiguousarray(total.T).reshape(1, S, DIM)
    if trace:
        kernel._last_results = r
    return out



# revision 18
# speedup vs baseline: 13.9846x; 13.9846x over previous
"""Causal self-attention (GQA 16q/4kv, rms_norm + rope + q_gain) on 8 trn2 cores.

Sharding: tensor-parallel over heads. Core c owns q-heads {2c, 2c+1} and kv-head
c//2. Each core computes its heads' attention output y_h^T [128, S] and a
partial projection partial_c^T [1024, S]; the host sums the 8 partials and
transposes back to [1, S, 1024].

Layout (per core, d-major = feature dim on partitions):
  Qt  [128, S] f32r  rows = 2 heads x 64 dims  (rms-normed, roped, gained)
  Kt2 [128, S] f32r  kv head duplicated to both partition halves
  V   [128, NKB, 80] bf16  seq-major tiles [128, 65] (col 64 = ones -> denom)
  St block = Kt_blk.T @ Qt_chunk -> [128 k, 2x512 q] psum, exp on ACT -> bf16
  diagonal blocks: column-restricted (q >= 128j) + 128-wide triangle mask
  AV: y^T[65, 512] += Vones_blk.T @ St_exp  (row 64 = softmax denom)
  proj of chunk c deferred + interleaved into attention of chunk c+1
  engines: PE matmuls, ACT exp/sq/rsqrt, DVE q-rope/normalize, Pool k-rope/
  V-copy/proj-copies/causal masks.
"""

import sys

for _p in ("/opt/trn_rl_repo",):
    if _p not in sys.path:
        sys.path.insert(0, _p)

import numpy as np
from contextlib import ExitStack

import concourse.bass as bass
import concourse.tile as tile
from concourse import mybir
from concourse.bass_utils import run_bass_kernel_spmd

F32 = mybir.dt.float32
F32R = mybir.dt.float32r
BF16 = mybir.dt.bfloat16
AF = mybir.ActivationFunctionType

DIM = 1024
NUM_HEADS = 16
NUM_KV_HEADS = 4
HD = 64
ROPE_BASE = 10000.0
RMS_EPS = 1.1920929e-07
N_CORES = 8

SCALE = 1.0 / 8.0  # 1/sqrt(64)


def build_nc(S: int, split: bool = True, iters: int = 1) -> bass.Bass:
    """Build the per-core SPMD Bass program for sequence length S (mult of 512)."""
    assert S % 512 == 0
    NCH = S // 512          # 512-wide seq chunks
    NKB = S // 128          # 128-wide k blocks

    nc = bass.Bass("TRN2", debug=False)

    xt_d = nc.declare_dram_parameter("xt", [DIM, S], F32R, isOutput=False)
    wqkv_d = nc.declare_dram_parameter("wqkv_t", [DIM, 256], F32R, isOutput=False)
    wproj_d = nc.declare_dram_parameter("wproj_t", [128, DIM], F32R, isOutput=False)
    cos_d = nc.declare_dram_parameter("cos4", [128, S], F32, isOutput=False)
    sin_d = nc.declare_dram_parameter("sin4s", [128, S], F32, isOutput=False)
    gain_d = nc.declare_dram_parameter("gain", [128, 1], F32, isOutput=False)
    out_d = nc.declare_dram_parameter("out", [DIM, S], BF16, isOutput=True)
    vt_dram = nc.dram_tensor("vt_scratch", [64, S], BF16)

    with tile.TileContext(nc) as tc, ExitStack() as ctx:
        res = ctx.enter_context(tc.tile_pool(name="res", bufs=1))
        xtp = ctx.enter_context(tc.tile_pool(name="xtp", bufs=2))
        ropep = ctx.enter_context(tc.tile_pool(name="ropep", bufs=2))
        rowp = ctx.enter_context(tc.tile_pool(name="rowp", bufs=4))
        vtp = ctx.enter_context(tc.tile_pool(name="vtp", bufs=2))
        sep = ctx.enter_context(tc.tile_pool(name="sep", bufs=3))
        outp = ctx.enter_context(tc.tile_pool(name="outp", bufs=1))
        pst = ctx.enter_context(tc.tile_pool(name="pst", bufs=2, space="PSUM"))
        pyp = ctx.enter_context(tc.tile_pool(name="pyp", bufs=1, space="PSUM"))
        pms = ctx.enter_context(tc.tile_pool(name="pms", bufs=2, space="PSUM"))

        # ---- resident tiles ----
        qt_sb = res.tile([128, S], F32R, tag="qt", name="qt_sb")
        kt2_sb = res.tile([128, S], F32R, tag="kt2", name="kt2_sb")
        yt_sb = res.tile([128, S], F32R, tag="yt", name="yt_sb")
        v_sb = res.tile([128, NKB, 80], BF16, tag="v", name="v_sb")
        wqkv_sb = res.tile([128, 8, 256], F32R, tag="wqkv", name="wqkv_sb")
        wproj_sb = res.tile([128, DIM], F32R, tag="wproj", name="wproj_sb")
        cos_sb = res.tile([128, S], F32, tag="cos", name="cos_sb")
        sin_sb = res.tile([128, S], F32, tag="sin", name="sin_sb")
        gain_sb = res.tile([128, 1], F32, tag="gain", name="gain_sb")
        ones_m = res.tile([128, 64], BF16, tag="onesm", name="ones_m")
        ones_r = res.tile([1, 64], F32R, tag="onesr", name="ones_r")
        const_sb = res.tile([128, 2], F32, tag="const", name="const_sb")
        fbq = res.tile([128, 512], F32, tag="fbq", name="fbq")
        fbk = res.tile([64, 512], F32, tag="fbk", name="fbk")
        rrn = res.tile([64, 1024], F32, tag="rrn", name="rrn")

        # ---- one-time setup ----
        # wqkv on the SP queue (first need); everything else on the ACT hwdge
        # queue so the SP queue reaches the chunk-0/1 xtile prefetches early.
        for dt in range(8):
            nc.sync.dma_start(out=wqkv_sb[:, dt, :], in_=wqkv_d[dt * 128:(dt + 1) * 128, :])
        nc.scalar.dma_start(out=cos_sb[:, 0:512], in_=cos_d[:, 0:512])
        nc.scalar.dma_start(out=sin_sb[:, 0:512], in_=sin_d[:, 0:512])
        nc.scalar.dma_start(out=gain_sb[:], in_=gain_d[:])
        nc.scalar.dma_start(out=cos_sb[:, 512:S], in_=cos_d[:, 512:S])
        nc.scalar.dma_start(out=sin_sb[:, 512:S], in_=sin_d[:, 512:S])
        nc.scalar.dma_start(out=wproj_sb[:], in_=wproj_d[:])
        nc.vector.memset(ones_m[:], 1.0)
        nc.vector.tensor_copy(ones_r[:], ones_m[0:1, :])
        nc.vector.memset(const_sb[:, 0:1], 0.0)
        nc.vector.memset(const_sb[:, 1:2], RMS_EPS)
        zb128 = const_sb[:, 0:1]          # zero bias, 128 partitions
        zb64 = const_sb[0:64, 0:1]
        # ones column of every V tile (col 64 of each 80-wide block)
        nc.vector.memset(v_sb[:, :, 64:65], 1.0)

        def _chunks():
            xtiles = {}

            def prefetch_x(c):
                if c >= NCH or c in xtiles:
                    return
                xtile = xtp.tile([128, 8, 512], F32R, tag="xt", name="xtile")
                xt_src = bass.AP(xt_d[:].tensor, c * 512,
                                 [[S, 128], [128 * S, 8], [1, 512]])
                nc.sync.dma_start(out=xtile[:], in_=xt_src)
                xtiles[c] = xtile

            def qkv_phase(c):
                s0 = c * 512
                sl = slice(s0, s0 + 512)
                prefetch_x(c + 1)
                xtile = xtiles.pop(c)

                # ---------- QKV projection (d-major): contraction over DIM ----------
                qkv_ps = pst.tile([128, 1024], F32, tag="st", name="qkv_ps")
                for dt in range(8):
                    nc.tensor.matmul(qkv_ps[:, 0:512], lhsT=wqkv_sb[:, dt, 0:128],
                                     rhs=xtile[:, dt, :], start=(dt == 0), stop=(dt == 7))
                for dt in range(8):
                    nc.tensor.matmul(qkv_ps[:, 512:1024], lhsT=wqkv_sb[:, dt, 128:256],
                                     rhs=xtile[:, dt, :], start=(dt == 0), stop=(dt == 7))

                # ---------- rms_norm factors (ACT squares -> PE ones-mm -> ACT rsqrt) ----------
                sq_q = ropep.tile([128, 512], BF16, tag="sqq", name="sq_q")
                nc.scalar.activation(sq_q[:], qkv_ps[:, 0:512], AF.Square, bias=zb128)
                sq_k = ropep.tile([64, 512], BF16, tag="sqk", name="sq_k")
                nc.scalar.activation(sq_k[:], qkv_ps[0:64, 512:1024], AF.Square, bias=zb64)

                msp_t = pms.tile([128, 512], F32, tag="b1", name="msp_t")
                nc.tensor.matmul(msp_t[0:64, :], lhsT=ones_m[0:64, :], rhs=sq_q[0:64, :],
                                 start=True, stop=True)
                nc.tensor.matmul(msp_t[64:128, :], lhsT=ones_m[64:128, :], rhs=sq_q[64:128, :],
                                 start=True, stop=True, tile_position=(64, 64))
                msk_t = pms.tile([128, 512], F32, tag="b1", name="msk_t")
                nc.tensor.matmul(msk_t[0:64, :], lhsT=ones_m[0:64, :], rhs=sq_k[:],
                                 start=True, stop=True)

                nc.scalar.activation(fbq[:], msp_t[:], AF.Sqrt, bias=const_sb[:, 1:2],
                                     scale=1.0 / HD)
                nc.scalar.activation(fbk[:], msk_t[0:64, :], AF.Sqrt,
                                     bias=const_sb[0:64, 1:2], scale=1.0 / HD)

                # ---------- rope + norm (DVE; qkv_ps readers front-loaded) ----------
                qsw = ropep.tile([128, 512], F32, tag="qsw", name="qsw")
                for qd in range(4):
                    nc.vector.tensor_copy(qsw[32 * qd:32 * qd + 32, :],
                                          qkv_ps[32 * (qd ^ 1):32 * (qd ^ 1) + 32, 0:512])
                t1 = ropep.tile([128, 512], F32, tag="t1", name="t1")
                nc.vector.tensor_mul(t1[:], qkv_ps[:, 0:512], cos_sb[:, sl])
                ksw = ropep.tile([64, 512], F32, tag="ksw", name="ksw")
                nc.vector.tensor_copy(ksw[0:32, :], qkv_ps[32:64, 512:1024])
                nc.vector.tensor_copy(ksw[32:64, :], qkv_ps[0:32, 512:1024])
                kt1 = ropep.tile([64, 512], F32, tag="kt1", name="kt1")
                nc.vector.tensor_mul(kt1[:], qkv_ps[0:64, 512:1024], cos_sb[0:64, sl])
                # V: bf16 copy, transpose to seq-major via HBM bounce
                vtt = vtp.tile([64, 512], BF16, tag="vt", name="vtt")
                nc.vector.tensor_copy(vtt[:], qkv_ps[64:128, 512:1024])
                # q combine
                t2 = ropep.tile([128, 512], F32, tag="t2", name="t2")
                nc.vector.tensor_mul(t2[:], qsw[:], sin_sb[:, sl])
                nc.vector.tensor_add(t1[:], t1[:], t2[:])
                nc.vector.reciprocal(fbq[:], fbq[:])
                nc.vector.tensor_scalar_mul(fbq[:], fbq[:], gain_sb[:, 0:1])
                nc.vector.tensor_mul(qt_sb[:, sl], t1[:], fbq[:])
                # k combine
                kt2t = ropep.tile([64, 512], F32, tag="kt2t", name="kt2t")
                nc.vector.tensor_mul(kt2t[:], ksw[:], sin_sb[0:64, sl])
                nc.vector.tensor_add(kt1[:], kt1[:], kt2t[:])
                nc.vector.reciprocal(fbk[:], fbk[:])
                nc.vector.tensor_mul(kt2_sb[0:64, sl], kt1[:], fbk[:])
                # duplicate kv head to partitions 64..127 (quadrant-aligned moves)
                nc.vector.tensor_copy(kt2_sb[64:96, sl], kt2_sb[0:32, sl])
                nc.vector.tensor_copy(kt2_sb[96:128, sl], kt2_sb[32:64, sl])

                nc.sync.dma_start(out=vt_dram[:, sl], in_=vtt[:])
                for j in range(4):
                    kb = 4 * c + j
                    nc.sync.dma_start_transpose(out=v_sb[:, kb, 0:64],
                                                in_=vt_dram[:, kb * 128:(kb + 1) * 128])

            def make_proj_tasks(qc, y0, y1):
                """Normalize chunk qc's attention output and return deferred
                projection tasks (run interleaved in the next chunk's kb loop)."""
                s0 = qc * 512
                sl = slice(s0, s0 + 512)

                # softmax denominators for both heads -> one row copy + f32r bcasts
                srow = rowp.tile([1, 1024], F32R, tag="row", name="srow")
                nc.vector.tensor_copy(srow[0:1, 0:512], y0[64:65, :])
                nc.vector.tensor_copy(srow[0:1, 512:1024], y1[64:65, :])
                rps0 = pms.tile([64, 512], F32, tag="b1", name="rps0")
                nc.tensor.matmul(rps0[:], lhsT=ones_r[:], rhs=srow[0:1, 0:512],
                                 start=True, stop=True)
                rps1 = pms.tile([64, 512], F32, tag="b1", name="rps1")
                nc.tensor.matmul(rps1[:], lhsT=ones_r[:], rhs=srow[0:1, 512:1024],
                                 start=True, stop=True)
                nc.vector.reciprocal(rrn[:, 0:512], rps0[:])
                nc.vector.reciprocal(rrn[:, 512:1024], rps1[:])
                nc.vector.tensor_mul(yt_sb[0:64, sl], y0[0:64, :], rrn[0:64, 0:512])
                nc.vector.tensor_mul(yt_sb[64:96, sl], y1[0:32, :], rrn[0:32, 512:1024])
                nc.vector.tensor_mul(yt_sb[96:128, sl], y1[32:64, :], rrn[32:64, 512:1024])

                ot_big = outp.tile([128, 8, 512], BF16, tag="ot", name="ot_big")

                def mk(ot):
                    def task():
                        po = pms.tile([128, 512], F32, tag="b1", name="po")
                        nc.tensor.matmul(po[:], lhsT=wproj_sb[:, ot * 128:(ot + 1) * 128],
                                         rhs=yt_sb[:, sl], start=True, stop=True)
                        nc.vector.tensor_copy(ot_big[:, ot, :], po[:])
                        if ot == 7:
                            out_dst = bass.AP(out_d[:].tensor, s0,
                                              [[S, 128], [128 * S, 8], [1, 512]])
                            nc.sync.dma_start(out=out_dst, in_=ot_big[:])
                    return task

                return [mk(ot) for ot in range(8)]

            def attn_kb(qc, kb, y0, y1):
                s0 = qc * 512
                nkb = 4 * (qc + 1)
                j = kb - 4 * qc
                qlo = 128 * j if j > 0 else 0
                first, last = (kb == 0), (kb == nkb - 1)

                stp = pst.tile([128, 1024], F32, tag="st", name="stp")
                nc.tensor.matmul(stp[:, qlo:512],
                                 lhsT=kt2_sb[0:64, kb * 128:(kb + 1) * 128],
                                 rhs=qt_sb[0:64, s0 + qlo:s0 + 512],
                                 start=True, stop=True, tile_position=(0, 0))
                nc.tensor.matmul(stp[:, 512 + qlo:1024],
                                 lhsT=kt2_sb[64:128, kb * 128:(kb + 1) * 128],
                                 rhs=qt_sb[64:128, s0 + qlo:s0 + 512],
                                 start=True, stop=True, tile_position=(64, 0))
                se = sep.tile([128, 1024], BF16, tag="se", name="se")
                if qlo == 0:
                    nc.scalar.activation(se[:], stp[:], AF.Exp, bias=zb128, scale=SCALE)
                else:
                    nc.scalar.activation(se[:, qlo:512], stp[:, qlo:512], AF.Exp,
                                         bias=zb128, scale=SCALE)
                    nc.scalar.activation(se[:, 512 + qlo:1024], stp[:, 512 + qlo:1024],
                                         AF.Exp, bias=zb128, scale=SCALE)
                if j >= 0:
                    # causal mask: only the 128-wide diagonal band needs it
                    se3 = bass.AP(se.tensor, se.offset + qlo,
                                  [se.ap[0], [512, 2], [1, 128]])
                    nc.gpsimd.affine_select(
                        out=se3, in_=se3, compare_op=mybir.AluOpType.is_ge,
                        fill=0.0, base=0, pattern=[[0, 2], [1, 128]],
                        channel_multiplier=-1)
                nc.tensor.matmul(y0[:, qlo:512], lhsT=v_sb[:, kb, 0:65],
                                 rhs=se[:, qlo:512], start=first, stop=last)
                nc.tensor.matmul(y1[:, qlo:512], lhsT=v_sb[:, kb, 0:65],
                                 rhs=se[:, 512 + qlo:1024], start=first, stop=last)

            # ---------------- main pipeline ----------------
            prefetch_x(0)
            qkv_phase(0)
            proj_tasks = []

            for c in range(NCH):
                nkb = 4 * (c + 1)
                y0 = pyp.tile([65, 512], F32, tag="y0", name="y0")
                y1 = pyp.tile([65, 512], F32, tag="y1", name="y1")
                # kb0/kb1 first: decouple the stp ring from next chunk's rope
                attn_kb(c, 0, y0, y1)
                if nkb > 1:
                    attn_kb(c, 1, y0, y1)
                if c + 1 < NCH:
                    qkv_phase(c + 1)
                # remaining kbs with deferred proj of chunk c-1 interleaved
                ntask = len(proj_tasks)
                done = 0
                for kb in range(2, nkb):
                    if done < ntask and kb >= 4 and (kb - 4) < ntask:
                        proj_tasks[kb - 4]()
                        done += 1
                    attn_kb(c, kb, y0, y1)
                for i in range(done, ntask):
                    proj_tasks[i]()
                proj_tasks = make_proj_tasks(c, y0, y1)

            for t in proj_tasks:
                t()

        if iters > 1:
            with tc.For_i(0, iters, 1) as _i:
                _chunks()
        else:
            _chunks()

    if split:
        split_multi_waits(nc)
    return nc


def split_multi_waits(nc, max_waits=1):
    """walrus's per-instruction sync encoding only fits one sem wait on some
    instruction types (e.g. the matmul LDWEIGHTS struct). Hoist extra waits
    onto same-engine NoOps inserted just before the instruction."""
    nid = [0]
    for fn in nc.m.functions:
        for blk in fn.blocks:
            out = []
            for inst in blk.instructions:
                si = inst.sync_info
                if si is not None and len(si.on_wait) > max_waits:
                    waits = list(si.on_wait)
                    for w in waits[:-max_waits]:
                        nop = mybir.InstNoOp(name=f"waitsplit-{nid[0]}", ins=[], outs=[])
                        nid[0] += 1
                        nop.engine = inst.engine
                        nop.sync_info = mybir.SyncInfo(on_wait=[w], on_update=[])
                        out.append(nop)
                    inst.sync_info = mybir.SyncInfo(on_wait=waits[-max_waits:],
                                                    on_update=list(si.on_update))
                out.append(inst)
            blk.instructions = out


def make_host_inputs(x, Wq, Wk, Wv, Wproj, q_gain, S):
    """Slice/transpose full inputs into per-core in_maps (host-side prep)."""
    xt = np.ascontiguousarray(x.reshape(S, DIM).T).astype(np.float32, copy=False)

    inv_freq = 1.0 / (ROPE_BASE ** (np.arange(0, HD, 2, dtype=np.float32) / HD))
    t = np.arange(S, dtype=np.float32)
    freqs = np.outer(t, inv_freq).astype(np.float32)        # [S, 32]
    cos_t = np.cos(freqs).T.astype(np.float32)              # [32, S]
    sin_t = np.sin(freqs).T.astype(np.float32)
    cos4 = np.ascontiguousarray(np.tile(cos_t, (4, 1)))     # [128, S]
    sin4s = np.ascontiguousarray(
        np.concatenate([sin_t, -sin_t, sin_t, -sin_t], axis=0))  # [128, S]

    in_maps = []
    for c in range(N_CORES):
        kv = c // 2
        wq_c = Wq[128 * c:128 * (c + 1), :]                 # [128, 1024]
        wk_c = Wk[64 * kv:64 * (kv + 1), :]                 # [64, 1024]
        wv_c = Wv[64 * kv:64 * (kv + 1), :]
        wqkv_t = np.ascontiguousarray(
            np.concatenate([wq_c, wk_c, wv_c], axis=0).T).astype(np.float32, copy=False)
        wproj_t = np.ascontiguousarray(
            Wproj[:, 128 * c:128 * (c + 1)].T).astype(np.float32, copy=False)
        gain = np.ascontiguousarray(
            np.repeat(q_gain[2 * c:2 * c + 2], 64).reshape(128, 1)).astype(np.float32, copy=False)
        in_maps.append({
            "xt": xt,
            "wqkv_t": wqkv_t,
            "wproj_t": wproj_t,
            "cos4": cos4,
            "sin4s": sin4s,
            "gain": gain,
        })
    return in_maps


_NC_CACHE = {}


def get_nc(S, iters=1):
    key = (S, iters)
    if key not in _NC_CACHE:
        _NC_CACHE[key] = build_nc(S, iters=iters)
    return _NC_CACHE[key]


def kernel(x, Wq, Wk, Wv, Wproj, q_gain, trace=False):
    x = np.asarray(x, dtype=np.float32)
    B, S, D = x.shape
    assert B == 1 and D == DIM
    in_maps = make_host_inputs(
        x, np.asarray(Wq, np.float32), np.asarray(Wk, np.float32),
        np.asarray(Wv, np.float32), np.asarray(Wproj, np.float32),
        np.asarray(q_gain, np.float32), S)

    nc = get_nc(S)
    r = run_bass_kernel_spmd(nc, in_maps, core_ids=list(range(N_CORES)), trace=trace)
    total = np.zeros((DIM, S), dtype=np.float32)
    for c in range(N_CORES):
        total += np.asarray(r.results[c]["out"]).astype(np.float32)
    out = np.ascontiguousarray(total.T).reshape(1, S, DIM)
    if trace:
        kernel._last_results = r
    return out
